# revision 24
# baseline (speedup 1.0000x reference)
"""CenterPNLoss on 8 TRN2 NeuronCores — fused rowsum + delta-chained psum.

Math: the reference builds two 8192x8192 distance matrices between
per-row class centers and all points, then does masked row reductions.
Both matrices have only <=1024 unique rows (one per identity g), and the
reference's reductions only ever need, per modality:
  rowsum[g]  = sum_j dist(c_g, x_j)            (full row sums), and
  S[g, h]    = sum_{j: t_j==h} dist(c_g, x_j)  for the <=8192 (g,h) pairs
               that the mask subtracts (h = t_j, g = row's center label).
The S terms touch only 8 points each -> computed EXACTLY on the host in
f64. The device only produces rowsum[g] per modality: a [1024, 8192]
distance sweep fully reduced on-chip.

Device pipeline (per core, a = c//4 center half, b = c%4 col quarter):
8 units of [128 centers x 2048 cols] over 2 resident psum tiles
(psA: units 0,2,4,6 = RGB m-blocks 0..3; psB: units 1,3,5,7 = IR).
  unit 0/1 (per tile): 4 fp8 DR bias matmuls (K_phys=1, stationary 2.0,
    nx hi+lo fp8 pair) open each bank start=True, then 4 fp8 DoubleRow
    mains (K_eff=256) accumulate -2 c[m0].x, stop=True.
  units 2..7: psum still holds nx - 2 c[m-1].x after the sqrt CONSUMED it
    (ACT only reads), so 4 DR matmuls with stationary -2(c[m]-c[m-1])
    re-close the banks (start=False, stop=True): nx rides along the whole
    chain and is paid for ONCE instead of per unit. 40 matmuls total, no
    DVE/GpSimd work, no nxb broadcast load.
  per unit: one ACT Sqrt over [128, 2048] with bias=nr[g] (f32 exact) and
    accum_out -> rowsum partial [128, 1] f32, fused in a single pass.
Output per core: racc [128, 8] f32 (m-block x modality rowsums), shipped
as cols 0:7 overlapped with the last unit + col 7 on the tail.
Host: assemble rowsums, compute S pairs + dist_pc in f64, form the loss.

Measured (fast clock state): compute phase is a constant 22.6us from the
first matmul (ACT chain at its 2.03us/unit floor -- sqrt is the only
per-element pass left, DVE/GpSimd idle; PE's 40 MMs at the sustained
427ns fp8-DR cadence hidden under it), bracketed by a ~3.1us DMA-start
latency (brc rides the sync queue first: lands ~0.7us earlier than via
the scalar queue, and it gates the first bias matmul) and a ~10us fixed
framework tail (racc DMA + 253 per-semaphore clears + barriers,
kernel-independent). HW exec 35.3-37.5us across runs (p-state/DMA
jitter) vs 42.8us for the previous DVE-reduce kernel.
Failed probes kept out: 1024-wide matmul dst (walrus s3d3 assert), PE
p-state warmup via dummy matmuls (sustained DR rate is throttle-set, not
ramp-set), in-place sqrt into psum (destroys the delta-chain operand),
tc.tile_wait_until on unit 1 to pin scheduler order (the resulting
schedule DEADLOCKS on hardware -- NRT_EXEC_UNIT_UNRECOVERABLE -- do not
retry; the brx duplicate-parameter dependency achieves the same ordering
safely and keeps ACT0's matmul-count threshold at 8 instead of 12).
"""

import sys
from contextlib import ExitStack

import numpy as np

sys.path.insert(0, "/opt/trn_rl_repo")

import concourse.bass as bass
import concourse.tile as tile
from concourse import bacc, mybir
from concourse.bass_utils import run_bass_kernel_spmd

N = 8192
D = 256
HALF = N // 2
NSEG = 1024
NCORES = 8
PW = 8                 # points per label (setup_inputs targets)
GC = 2048              # data columns per core
MB = 4                 # center blocks (of 128) per core
BANK_F = 512           # psum bank width in f32 (matmul dst cap)

FP8 = mybir.dt.float8e4
DR = mybir.MatmulPerfMode.DoubleRow

_nc_cache: dict = {}
last_result = None  # BassKernelResults of the most recent run (for test.py)


def build_nc():
    """One-core SPMD program: fp8 operands -> [128, 8] f32 rowsum shard."""
    f32 = mybir.dt.float32
    bf16 = mybir.dt.bfloat16
    Sqrt = mybir.ActivationFunctionType.Sqrt

    nc = bacc.Bacc()
    # xr[k, t*2048 + i*1024 + j] = x[b*2048 + t*1024 + j, i*128 + k]
    xr_d = nc.declare_dram_parameter("xr", [128, 4096], FP8, isOutput=False)
    # lh[k, mod*1024 + i*512 + g]: g block 0 = -2*c_mod[a*512 + g, 128i+k],
    # g block m>0 = -2*(c[a*512+128m+g] - c[a*512+128(m-1)+g]) (deltas)
    lh_d = nc.declare_dram_parameter("lh", [128, 2048], FP8, isOutput=False)
    # brc[0, 0:256] = 2.0 (bias stationary); [0, 256+i*2048+j]: i=0 ->
    # nx_hi[col]/2, i=1 -> nx_lo[col]/2
    brc_d = nc.declare_dram_parameter("brc", [1, 4352], FP8, isOutput=False)
    # brx: copy of brc whose DMA lands after the xr stream -- unit 1's bias
    # reads it so the scheduler cannot hoist those matmuls ahead of unit 0's
    # mains (that hoist inflates ACT0's matmul-count threshold 8 -> 12)
    brx_d = nc.declare_dram_parameter("brx", [1, 4352], FP8, isOutput=False)
    # nr[p, m_local*2+mod] = ||center[a*512 + m_local*128 + p]||^2 (f32 exact)
    nr_d = nc.declare_dram_parameter("nr", [128, 8], f32, isOutput=False)
    racc_d = nc.declare_dram_parameter("racc", [128, 9], f32, isOutput=True)

    with tile.TileContext(nc) as tc, ExitStack() as ctx:
        const = ctx.enter_context(tc.tile_pool(name="const", bufs=1))
        psum = ctx.enter_context(tc.tile_pool(name="psum", bufs=1, space="PSUM"))

        xr = const.tile([128, 4096], FP8, tag="xr")
        lh = const.tile([128, 2048], FP8, tag="lh")
        brc = const.tile([1, 4352], FP8, tag="brc")
        brx = const.tile([1, 4352], FP8, tag="brx")
        nr_t = const.tile([128, 8], f32, tag="nr")
        warm = const.tile([1, 8], f32, tag="warm")
        racc = const.tile([128, 9], f32, tag="racc")
        d_t = [
            const.tile([128, 2048], bf16, tag=f"d{i}", name=f"d{i}")
            for i in range(2)
        ]
        ps = [
            psum.tile([128, 2048], f32, tag=f"ps{i}", name=f"ps{i}")
            for i in range(2)
        ]

        # brc rides the sync queue FIRST: its packets land ~0.7us earlier
        # than on the scalar queue (whose sequencer is busy with the ACT
        # table loads), and brc gates the first bias matmul = the whole
        # chain's start. xr streams behind it; lh/nr go on the scalar group.
        nc.sync.dma_start(out=brc[:], in_=brc_d[:, :])
        nc.sync.dma_start(out=xr[:, 0:2048], in_=xr_d[:, 0:2048])
        nc.sync.dma_start(out=xr[:, 2048:4096], in_=xr_d[:, 2048:4096])
        nc.sync.dma_start(out=brx[:], in_=brx_d[:, :])
        nc.scalar.dma_start(out=lh[:], in_=lh_d[:, :])
        nc.scalar.dma_start(out=nr_t[:], in_=nr_d[:, :])
        # loads the ACT Sqrt table while the big DMAs are still in flight;
        # fed from brc (the first transfer to land) so the 2x1.3us table
        # loads clear the ACT queue long before the first real sqrt
        nc.scalar.activation(warm[:], brc[:, 0:8], Sqrt)

        def emit_bias(pu, src_t):
            bc3 = src_t[:, 0:256].rearrange("p (i g) -> p i g", i=2)
            br3 = src_t[:, 256:4352].rearrange("p (i n) -> p i n", i=2)
            for t in range(4):
                nc.tensor.matmul(
                    pu[:, t * BANK_F : (t + 1) * BANK_F],
                    bc3,
                    br3[:, :, t * BANK_F : (t + 1) * BANK_F],
                    start=True, stop=False, perf_mode=DR,
                )

        def emit_mains(pu, m_local, mod, first):
            lt = lh[:, mod * 1024 : (mod + 1) * 1024].rearrange(
                "p (i g) -> p i g", i=2
            )[:, :, m_local * 128 : (m_local + 1) * 128]
            xr5 = xr[:].rearrange(
                "p (T i jb j) -> p T jb i j", T=2, i=2, jb=2
            )
            for t in range(4):
                nc.tensor.matmul(
                    pu[:, t * BANK_F : (t + 1) * BANK_F],
                    lt,
                    xr5[:, t // 2, t % 2],
                    start=False, stop=True, perf_mode=DR,
                    skip_group_check=not first,
                )

        def emit_act(pu, u):
            nc.scalar.activation(
                d_t[u % 2][:], pu[:], Sqrt,
                bias=nr_t[:, u : u + 1], scale=1.0,
                accum_out=racc[:, u : u + 1],
            )

        # unit 0 (psA, RGB m0) completes first so the ACT chain starts
        # after 8 real matmuls; unit 1 (psB, IR m0) follows
        emit_bias(ps[0], brc)
        emit_mains(ps[0], 0, 0, first=True)
        emit_act(ps[0], 0)
        emit_bias(ps[1], brx)
        emit_mains(ps[1], 0, 1, first=True)
        # unit 1's sqrt is split in halves: the first half only needs the
        # first two psum banks, so it fills the ACT-queue hole between
        # ACT0's end and the full 16-matmul drain (~1.5us earlier start).
        # Halves accumulate into racc cols 1 and 8; the host sums them.
        nc.scalar.activation(
            d_t[1][:, 0:1024], ps[1][:, 0:1024], Sqrt,
            bias=nr_t[:, 1:2], scale=1.0, accum_out=racc[:, 1:2],
        )
        nc.scalar.activation(
            d_t[1][:, 1024:2048], ps[1][:, 1024:2048], Sqrt,
            bias=nr_t[:, 1:2], scale=1.0, accum_out=racc[:, 8:9],
        )
        # units 2..7: delta mains re-close the still-loaded banks
        for u in range(2, 2 * MB):
            m_local, mod = u // 2, u % 2
            emit_mains(ps[u % 2], m_local, mod, first=False)
            emit_act(ps[u % 2], u)
            if u == 6:
                # most of the output ships while the last unit computes;
                # only the final column rides the critical tail
                nc.sync.dma_start(out=racc_d[:, 0:7], in_=racc[:, 0:7])
        # issued from the ACT engine's own queue: runs in-order right after
        # the last accumulator read, skipping a cross-engine semaphore hop
        nc.scalar.dma_start(out=racc_d[:, 7:9], in_=racc[:, 7:9])
    nc.finalize()
    return nc


def _seg_mean(x_half: np.ndarray, t_half: np.ndarray):
    """f64 segment mean matching jax.ops.segment_sum + max(count,1) divide."""
    cnt = np.bincount(t_half, minlength=NSEG)
    sums = np.zeros((NSEG, D), np.float64)
    order = np.argsort(t_half, kind="stable")
    xs = x_half[order].astype(np.float64)
    ts_sorted = t_half[order]
    present = np.nonzero(cnt)[0]
    if len(present):
        starts = np.searchsorted(ts_sorted, present)
        sums[present] = np.add.reduceat(xs, starts, axis=0)
    return sums / np.maximum(cnt, 1)[:, None], cnt


def prepare(inputs: np.ndarray, targets: np.ndarray):
    """Host marshaling: centers, fp8 DoubleRow operand layouts, in_maps."""
    fp8_np = mybir.dt.np(FP8)
    x = np.asarray(inputs, np.float32)
    t = np.asarray(targets)
    centerR64, _ = _seg_mean(x[:HALF], t[:HALF])
    centerI64, _ = _seg_mean(x[HALF:], t[HALF:])
    centerR = centerR64.astype(np.float32)
    centerI = centerI64.astype(np.float32)
    nrR64 = np.sum(centerR.astype(np.float64) ** 2, axis=1)
    nrI64 = np.sum(centerI.astype(np.float64) ** 2, axis=1)
    n_x64 = np.sum(x.astype(np.float64) ** 2, axis=1)

    cnt_all = np.bincount(t, minlength=NSEG)
    assert cnt_all.min() == cnt_all.max() == PW, "kernel hardcodes 8 pts/label"

    # nx ~= 2*hi + 2*lo with hi, lo in fp8 (e4m3 max 448 forces the /2)
    nxh = (n_x64 / 2.0).astype(fp8_np)
    nxl = ((n_x64 - 2.0 * nxh.astype(np.float64)) / 2.0).astype(fp8_np)

    def mk_lh(center, a):
        c = center[a * 512 : (a + 1) * 512]           # [512, 256]
        w = np.empty((512, D), np.float32)
        w[0:128] = -2.0 * c[0:128]
        for m in range(1, MB):
            w[m * 128 : (m + 1) * 128] = -2.0 * (
                c[m * 128 : (m + 1) * 128] - c[(m - 1) * 128 : m * 128]
            )
        v = w.reshape(512, 2, 128)                    # [g, i, k]
        return np.ascontiguousarray(
            v.transpose(2, 1, 0).reshape(128, 1024)
        ).astype(fp8_np)

    lhs = [
        np.concatenate([mk_lh(centerR, a), mk_lh(centerI, a)], axis=1)
        for a in range(2)
    ]
    nrs = []
    for a in range(2):
        nr_t = np.zeros((128, 8), np.float32)
        for m_local in range(MB):
            sl = slice(a * 512 + m_local * 128, a * 512 + m_local * 128 + 128)
            nr_t[:, m_local * 2] = nrR64[sl]
            nr_t[:, m_local * 2 + 1] = nrI64[sl]
        nrs.append(nr_t)

    in_maps = []
    for c in range(NCORES):
        a, b = c // 4, c % 4
        xc = x[b * GC : (b + 1) * GC]         # [2048, 256], natural order
        xr = np.empty((128, 2, 2, 1024), fp8_np)
        for tt in range(2):
            v = xc[tt * 1024 : (tt + 1) * 1024].reshape(1024, 2, 128)
            xr[:, tt] = v.transpose(2, 1, 0)  # [k, i, j]
        brc = np.empty((1, 4352), fp8_np)
        brc[0, :256] = np.float32(2.0)
        brc[0, 256 : 256 + 2048] = nxh[b * GC : (b + 1) * GC]
        brc[0, 256 + 2048 :] = nxl[b * GC : (b + 1) * GC]
        in_maps.append(
            {
                "xr": np.ascontiguousarray(xr.reshape(128, 4096)),
                "lh": lhs[a],
                "brc": brc,
                "brx": brc,
                "nr": nrs[a],
            }
        )

    host = dict(
        centerR64=centerR64, centerI64=centerI64,
        nrR64=nrR64, nrI64=nrI64, n_x64=n_x64,
        cnt_all=cnt_all, targets=t, x=x,
    )
    return in_maps, host


def finish(core_outs, host) -> np.float32:
    """Assemble rowsums; S pairs + dist_pc exactly in f64; form the loss."""
    t = host["targets"]
    cnt = host["cnt_all"]
    cR, cI = host["centerR64"], host["centerI64"]
    nrR, nrI = host["nrR64"], host["nrI64"]
    nx = host["n_x64"]
    x = host["x"].astype(np.float64)

    rowsumR = np.zeros(NSEG, np.float64)
    rowsumI = np.zeros(NSEG, np.float64)
    for c in range(NCORES):
        a = c // 4
        racc = core_outs[c].astype(np.float64)    # [128, 9]
        racc[:, 1] += racc[:, 8]                  # unit 1's split halves
        for m_local in range(MB):
            rows = slice(a * 512 + m_local * 128, a * 512 + m_local * 128 + 128)
            rowsumR[rows] += racc[:, m_local * 2]
            rowsumI[rows] += racc[:, m_local * 2 + 1]

    # masked-out terms S[g, h] = sum_{j: t_j==h} dist(c_g, x_j), exact f64,
    # only for the (row center, row label) pairs the reference subtracts
    gqR = t[np.arange(N) % HALF]
    gqI = t[HALF + (np.arange(N) % HALF)]
    order = np.argsort(t, kind="stable")
    pts_by_label = x[order].reshape(NSEG, PW, D)      # label-major points
    nx_by_label = nx[order].reshape(NSEG, PW)

    def s_pairs(gq, centers, nrc):
        pair_ids = gq.astype(np.int64) * NSEG + t
        uniq, inv = np.unique(pair_ids, return_inverse=True)
        ug, uh = uniq // NSEG, uniq % NSEG
        cc = centers[ug]                               # [u, D]
        pp = pts_by_label[uh]                          # [u, PW, D]
        d2 = (
            nrc[ug][:, None]
            + nx_by_label[uh]
            - 2.0 * np.einsum("ud,upd->up", cc, pp)
        )
        s_u = np.sqrt(np.maximum(d2, 1e-12)).sum(axis=1)
        return s_u[inv]                                # [N]

    SR = s_pairs(gqR, cR, nrR)
    SI = s_pairs(gqI, cI, nrI)
    a_w = 1.0 / (N - cnt[t]).astype(np.float64)
    sumR = float(np.sum(a_w * (rowsumR[gqR] - SR)))
    sumI = float(np.sum(a_w * (rowsumI[gqI] - SI)))

    diff = cR[t[:HALF]] - cI[t[HALF:]]
    s_pc = float(np.sum(np.sqrt(np.sum(diff * diff, axis=1))))
    return np.float32(s_pc / (sumR + sumI - s_pc))


def kernel(inputs: np.ndarray, targets: np.ndarray) -> np.ndarray:
    global last_result
    in_maps, host = prepare(inputs, targets)
    if "nc" not in _nc_cache:
        _nc_cache["nc"] = build_nc()
    nc = _nc_cache["nc"]
    res = run_bass_kernel_spmd(nc, in_maps, list(range(NCORES)))
    last_result = res
    outs = [res.results[c]["racc"] for c in range(NCORES)]
    return finish(outs, host)


# revision 25
# speedup vs baseline: 1.0851x; 1.0851x over previous
"""CenterPNLoss on 8 TRN2 NeuronCores — fused rowsum + delta-chained psum.

Math: the reference builds two 8192x8192 distance matrices between
per-row class centers and all points, then does masked row reductions.
Both matrices have only <=1024 unique rows (one per identity g), and the
reference's reductions only ever need, per modality:
  rowsum[g]  = sum_j dist(c_g, x_j)            (full row sums), and
  S[g, h]    = sum_{j: t_j==h} dist(c_g, x_j)  for the <=8192 (g,h) pairs
               that the mask subtracts (h = t_j, g = row's center label).
The S terms touch only 8 points each -> computed EXACTLY on the host in
f64. The device only produces rowsum[g] per modality: a [1024, 8192]
distance sweep fully reduced on-chip.

Device pipeline (per core, a = c//4 center half, b = c%4 col quarter):
8 units of [128 centers x 2048 cols] over 2 resident psum tiles
(psA: units 0,2,4,6 = RGB m-blocks 0..3; psB: units 1,3,5,7 = IR).
  unit 0/1 (per tile): 4 fp8 DR bias matmuls (K_phys=1, stationary 2.0,
    nx hi+lo fp8 pair) open each bank start=True, then 4 fp8 DoubleRow
    mains (K_eff=256) accumulate -2 c[m0].x, stop=True.
  units 2..7: psum still holds nx - 2 c[m-1].x after the sqrt CONSUMED it
    (ACT only reads), so 4 DR matmuls with stationary -2(c[m]-c[m-1])
    re-close the banks (start=False, stop=True): nx rides along the whole
    chain and is paid for ONCE instead of per unit. 40 matmuls total, no
    DVE/GpSimd work, no nxb broadcast load.
  per unit: one ACT Sqrt over [128, 2048] with bias=nr[g] (f32 exact) and
    accum_out -> rowsum partial [128, 1] f32, fused in a single pass.
Output per core: racc [128, 8] f32 (m-block x modality rowsums), shipped
as cols 0:7 overlapped with the last unit + col 7 on the tail.
Host: assemble rowsums, compute S pairs + dist_pc in f64, form the loss.

Measured (fast clock state): compute phase is a constant 22.6us from the
first matmul (ACT chain at its 2.03us/unit floor -- sqrt is the only
per-element pass left, DVE/GpSimd idle; PE's 40 MMs at the sustained
427ns fp8-DR cadence hidden under it), bracketed by a ~3.1us DMA-start
latency (brc rides the sync queue first: lands ~0.7us earlier than via
the scalar queue, and it gates the first bias matmul) and a ~10us fixed
framework tail (racc DMA + 253 per-semaphore clears + barriers,
kernel-independent). HW exec 35.3-37.5us across runs (p-state/DMA
jitter) vs 42.8us for the previous DVE-reduce kernel.
Failed probes kept out: 1024-wide matmul dst (walrus s3d3 assert), PE
p-state warmup via dummy matmuls (sustained DR rate is throttle-set, not
ramp-set), in-place sqrt into psum (destroys the delta-chain operand),
tc.tile_wait_until on unit 1 to pin scheduler order (the resulting
schedule DEADLOCKS on hardware -- NRT_EXEC_UNIT_UNRECOVERABLE -- do not
retry; the brx duplicate-parameter dependency achieves the same ordering
safely and keeps ACT0's matmul-count threshold at 8 instead of 12).
"""

import sys
from contextlib import ExitStack

import numpy as np

sys.path.insert(0, "/opt/trn_rl_repo")

import concourse.bass as bass
import concourse.tile as tile
from concourse import bacc, mybir
from concourse.bass_utils import run_bass_kernel_spmd

N = 8192
D = 256
HALF = N // 2
NSEG = 1024
NCORES = 8
PW = 8                 # points per label (setup_inputs targets)
GC = 2048              # data columns per core
MB = 4                 # center blocks (of 128) per core
BANK_F = 512           # psum bank width in f32 (matmul dst cap)

FP8 = mybir.dt.float8e4
DR = mybir.MatmulPerfMode.DoubleRow

_nc_cache: dict = {}
last_result = None  # BassKernelResults of the most recent run (for test.py)


def build_nc():
    """One-core SPMD program: fp8 operands -> [128, 8] f32 rowsum shard."""
    f32 = mybir.dt.float32
    bf16 = mybir.dt.bfloat16
    Sqrt = mybir.ActivationFunctionType.Sqrt

    nc = bacc.Bacc()
    # xr[k, t*2048 + i*1024 + j] = x[b*2048 + t*1024 + j, i*128 + k]
    xr_d = nc.declare_dram_parameter("xr", [128, 4096], FP8, isOutput=False)
    # lh[k, mod*1024 + i*512 + g]: g block 0 = -2*c_mod[a*512 + g, 128i+k],
    # g block m>0 = -2*(c[a*512+128m+g] - c[a*512+128(m-1)+g]) (deltas)
    lh_d = nc.declare_dram_parameter("lh", [128, 2048], FP8, isOutput=False)
    # brc[0, 0:256] = 2.0 (bias stationary); [0, 256+i*2048+j]: i=0 ->
    # nx_hi[col]/2, i=1 -> nx_lo[col]/2
    brc_d = nc.declare_dram_parameter("brc", [1, 4352], FP8, isOutput=False)
    # brx: copy of brc whose DMA lands after the xr stream -- unit 1's bias
    # reads it so the scheduler cannot hoist those matmuls ahead of unit 0's
    # mains (that hoist inflates ACT0's matmul-count threshold 8 -> 12)
    brx_d = nc.declare_dram_parameter("brx", [1, 4352], FP8, isOutput=False)
    # nr[p, m_local*2+mod] = ||center[a*512 + m_local*128 + p]||^2 (f32 exact)
    nr_d = nc.declare_dram_parameter("nr", [128, 8], f32, isOutput=False)
    racc_d = nc.declare_dram_parameter("racc", [128, 8], f32, isOutput=True)

    with tile.TileContext(nc) as tc, ExitStack() as ctx:
        const = ctx.enter_context(tc.tile_pool(name="const", bufs=1))
        psum = ctx.enter_context(tc.tile_pool(name="psum", bufs=1, space="PSUM"))

        xr = const.tile([128, 4096], FP8, tag="xr")
        lh = const.tile([128, 2048], FP8, tag="lh")
        brc = const.tile([1, 4352], FP8, tag="brc")
        brx = const.tile([1, 4352], FP8, tag="brx")
        nr_t = const.tile([128, 8], f32, tag="nr")
        warm = const.tile([1, 8], f32, tag="warm")
        racc = const.tile([128, 8], f32, tag="racc")
        d_t = [
            const.tile([128, 2048], bf16, tag=f"d{i}", name=f"d{i}")
            for i in range(2)
        ]
        ps = [
            psum.tile([128, 2048], f32, tag=f"ps{i}", name=f"ps{i}")
            for i in range(2)
        ]

        # brc rides the sync queue FIRST: its packets land ~0.7us earlier
        # than on the scalar queue (whose sequencer is busy with the ACT
        # table loads), and brc gates the first bias matmul = the whole
        # chain's start. xr streams behind it; lh/nr go on the scalar group.
        nc.sync.dma_start(out=brc[:], in_=brc_d[:, :])
        nc.sync.dma_start(out=xr[:, 0:2048], in_=xr_d[:, 0:2048])
        nc.sync.dma_start(out=xr[:, 2048:4096], in_=xr_d[:, 2048:4096])
        nc.sync.dma_start(out=brx[:], in_=brx_d[:, :])
        nc.scalar.dma_start(out=lh[:], in_=lh_d[:, :])
        nc.scalar.dma_start(out=nr_t[:], in_=nr_d[:, :])
        # loads the ACT Sqrt table while the big DMAs are still in flight;
        # fed from brc (the first transfer to land) so the 2x1.3us table
        # loads clear the ACT queue long before the first real sqrt
        nc.scalar.activation(warm[:], brc[:, 0:8], Sqrt)

        def emit_bias(pu, src_t):
            bc3 = src_t[:, 0:256].rearrange("p (i g) -> p i g", i=2)
            br3 = src_t[:, 256:4352].rearrange("p (i n) -> p i n", i=2)
            for t in range(4):
                nc.tensor.matmul(
                    pu[:, t * BANK_F : (t + 1) * BANK_F],
                    bc3,
                    br3[:, :, t * BANK_F : (t + 1) * BANK_F],
                    start=True, stop=False, perf_mode=DR,
                )

        def emit_mains(pu, m_local, mod, first):
            lt = lh[:, mod * 1024 : (mod + 1) * 1024].rearrange(
                "p (i g) -> p i g", i=2
            )[:, :, m_local * 128 : (m_local + 1) * 128]
            xr5 = xr[:].rearrange(
                "p (T i jb j) -> p T jb i j", T=2, i=2, jb=2
            )
            for t in range(4):
                nc.tensor.matmul(
                    pu[:, t * BANK_F : (t + 1) * BANK_F],
                    lt,
                    xr5[:, t // 2, t % 2],
                    start=False, stop=True, perf_mode=DR,
                    skip_group_check=not first,
                )

        def emit_act(pu, u):
            nc.scalar.activation(
                d_t[u % 2][:], pu[:], Sqrt,
                bias=nr_t[:, u : u + 1], scale=1.0,
                accum_out=racc[:, u : u + 1],
            )

        # unit 0 (psA, RGB m0) completes first so the ACT chain starts
        # after 8 real matmuls; unit 1 (psB, IR m0) follows
        emit_bias(ps[0], brc)
        emit_mains(ps[0], 0, 0, first=True)
        emit_act(ps[0], 0)
        emit_bias(ps[1], brx)
        emit_mains(ps[1], 0, 1, first=True)
        emit_act(ps[1], 1)
        # units 2..7: delta mains re-close the still-loaded banks
        for u in range(2, 2 * MB):
            m_local, mod = u // 2, u % 2
            emit_mains(ps[u % 2], m_local, mod, first=False)
            emit_act(ps[u % 2], u)
            if u == 6:
                # most of the output ships while the last unit computes;
                # only the final column rides the critical tail
                nc.sync.dma_start(out=racc_d[:, 0:7], in_=racc[:, 0:7])
        # issued from the ACT engine's own queue: runs in-order right after
        # the last accumulator read, skipping a cross-engine semaphore hop
        nc.scalar.dma_start(out=racc_d[:, 7:8], in_=racc[:, 7:8])
    nc.finalize()
    return nc


def _seg_mean(x_half: np.ndarray, t_half: np.ndarray):
    """f64 segment mean matching jax.ops.segment_sum + max(count,1) divide."""
    cnt = np.bincount(t_half, minlength=NSEG)
    sums = np.zeros((NSEG, D), np.float64)
    order = np.argsort(t_half, kind="stable")
    xs = x_half[order].astype(np.float64)
    ts_sorted = t_half[order]
    present = np.nonzero(cnt)[0]
    if len(present):
        starts = np.searchsorted(ts_sorted, present)
        sums[present] = np.add.reduceat(xs, starts, axis=0)
    return sums / np.maximum(cnt, 1)[:, None], cnt


def prepare(inputs: np.ndarray, targets: np.ndarray):
    """Host marshaling: centers, fp8 DoubleRow operand layouts, in_maps."""
    fp8_np = mybir.dt.np(FP8)
    x = np.asarray(inputs, np.float32)
    t = np.asarray(targets)
    centerR64, _ = _seg_mean(x[:HALF], t[:HALF])
    centerI64, _ = _seg_mean(x[HALF:], t[HALF:])
    centerR = centerR64.astype(np.float32)
    centerI = centerI64.astype(np.float32)
    nrR64 = np.sum(centerR.astype(np.float64) ** 2, axis=1)
    nrI64 = np.sum(centerI.astype(np.float64) ** 2, axis=1)
    n_x64 = np.sum(x.astype(np.float64) ** 2, axis=1)

    cnt_all = np.bincount(t, minlength=NSEG)
    assert cnt_all.min() == cnt_all.max() == PW, "kernel hardcodes 8 pts/label"

    # nx ~= 2*hi + 2*lo with hi, lo in fp8 (e4m3 max 448 forces the /2)
    nxh = (n_x64 / 2.0).astype(fp8_np)
    nxl = ((n_x64 - 2.0 * nxh.astype(np.float64)) / 2.0).astype(fp8_np)

    def mk_lh(center, a):
        c = center[a * 512 : (a + 1) * 512]           # [512, 256]
        w = np.empty((512, D), np.float32)
        w[0:128] = -2.0 * c[0:128]
        for m in range(1, MB):
            w[m * 128 : (m + 1) * 128] = -2.0 * (
                c[m * 128 : (m + 1) * 128] - c[(m - 1) * 128 : m * 128]
            )
        v = w.reshape(512, 2, 128)                    # [g, i, k]
        return np.ascontiguousarray(
            v.transpose(2, 1, 0).reshape(128, 1024)
        ).astype(fp8_np)

    lhs = [
        np.concatenate([mk_lh(centerR, a), mk_lh(centerI, a)], axis=1)
        for a in range(2)
    ]
    nrs = []
    for a in range(2):
        nr_t = np.zeros((128, 8), np.float32)
        for m_local in range(MB):
            sl = slice(a * 512 + m_local * 128, a * 512 + m_local * 128 + 128)
            nr_t[:, m_local * 2] = nrR64[sl]
            nr_t[:, m_local * 2 + 1] = nrI64[sl]
        nrs.append(nr_t)

    in_maps = []
    for c in range(NCORES):
        a, b = c // 4, c % 4
        xc = x[b * GC : (b + 1) * GC]         # [2048, 256], natural order
        xr = np.empty((128, 2, 2, 1024), fp8_np)
        for tt in range(2):
            v = xc[tt * 1024 : (tt + 1) * 1024].reshape(1024, 2, 128)
            xr[:, tt] = v.transpose(2, 1, 0)  # [k, i, j]
        brc = np.empty((1, 4352), fp8_np)
        brc[0, :256] = np.float32(2.0)
        brc[0, 256 : 256 + 2048] = nxh[b * GC : (b + 1) * GC]
        brc[0, 256 + 2048 :] = nxl[b * GC : (b + 1) * GC]
        in_maps.append(
            {
                "xr": np.ascontiguousarray(xr.reshape(128, 4096)),
                "lh": lhs[a],
                "brc": brc,
                "brx": brc,
                "nr": nrs[a],
            }
        )

    host = dict(
        centerR64=centerR64, centerI64=centerI64,
        nrR64=nrR64, nrI64=nrI64, n_x64=n_x64,
        cnt_all=cnt_all, targets=t, x=x,
    )
    return in_maps, host


def finish(core_outs, host) -> np.float32:
    """Assemble rowsums; S pairs + dist_pc exactly in f64; form the loss."""
    t = host["targets"]
    cnt = host["cnt_all"]
    cR, cI = host["centerR64"], host["centerI64"]
    nrR, nrI = host["nrR64"], host["nrI64"]
    nx = host["n_x64"]
    x = host["x"].astype(np.float64)

    rowsumR = np.zeros(NSEG, np.float64)
    rowsumI = np.zeros(NSEG, np.float64)
    for c in range(NCORES):
        a = c // 4
        racc = core_outs[c].astype(np.float64)    # [128, 8]
        for m_local in range(MB):
            rows = slice(a * 512 + m_local * 128, a * 512 + m_local * 128 + 128)
            rowsumR[rows] += racc[:, m_local * 2]
            rowsumI[rows] += racc[:, m_local * 2 + 1]

    # masked-out terms S[g, h] = sum_{j: t_j==h} dist(c_g, x_j), exact f64,
    # only for the (row center, row label) pairs the reference subtracts
    gqR = t[np.arange(N) % HALF]
    gqI = t[HALF + (np.arange(N) % HALF)]
    order = np.argsort(t, kind="stable")
    pts_by_label = x[order].reshape(NSEG, PW, D)      # label-major points
    nx_by_label = nx[order].reshape(NSEG, PW)

    def s_pairs(gq, centers, nrc):
        pair_ids = gq.astype(np.int64) * NSEG + t
        uniq, inv = np.unique(pair_ids, return_inverse=True)
        ug, uh = uniq // NSEG, uniq % NSEG
        cc = centers[ug]                               # [u, D]
        pp = pts_by_label[uh]                          # [u, PW, D]
        d2 = (
            nrc[ug][:, None]
            + nx_by_label[uh]
            - 2.0 * np.einsum("ud,upd->up", cc, pp)
        )
        s_u = np.sqrt(np.maximum(d2, 1e-12)).sum(axis=1)
        return s_u[inv]                                # [N]

    SR = s_pairs(gqR, cR, nrR)
    SI = s_pairs(gqI, cI, nrI)
    a_w = 1.0 / (N - cnt[t]).astype(np.float64)
    sumR = float(np.sum(a_w * (rowsumR[gqR] - SR)))
    sumI = float(np.sum(a_w * (rowsumI[gqI] - SI)))

    diff = cR[t[:HALF]] - cI[t[HALF:]]
    s_pc = float(np.sum(np.sqrt(np.sum(diff * diff, axis=1))))
    return np.float32(s_pc / (sumR + sumI - s_pc))


def kernel(inputs: np.ndarray, targets: np.ndarray) -> np.ndarray:
    global last_result
    in_maps, host = prepare(inputs, targets)
    if "nc" not in _nc_cache:
        _nc_cache["nc"] = build_nc()
    nc = _nc_cache["nc"]
    res = run_bass_kernel_spmd(nc, in_maps, list(range(NCORES)))
    last_result = res
    outs = [res.results[c]["racc"] for c in range(NCORES)]
    return finish(outs, host)


# revision 26
# speedup vs baseline: 1.1095x; 1.0225x over previous
"""CenterPNLoss on 8 TRN2 NeuronCores — fused rowsum + delta-chained psum.

Math: the reference builds two 8192x8192 distance matrices between
per-row class centers and all points, then does masked row reductions.
Both matrices have only <=1024 unique rows (one per identity g), and the
reference's reductions only ever need, per modality:
  rowsum[g]  = sum_j dist(c_g, x_j)            (full row sums), and
  S[g, h]    = sum_{j: t_j==h} dist(c_g, x_j)  for the <=8192 (g,h) pairs
               that the mask subtracts (h = t_j, g = row's center label).
The S terms touch only 8 points each -> computed EXACTLY on the host in
f64. The device only produces rowsum[g] per modality: a [1024, 8192]
distance sweep fully reduced on-chip.

Device pipeline (per core, a = c//4 center half, b = c%4 col quarter):
8 units of [128 centers x 2048 cols] over 2 resident psum tiles
(psA: units 0,2,4,6 = RGB m-blocks 0..3; psB: units 1,3,5,7 = IR).
  unit 0/1 (per tile): 4 fp8 DR bias matmuls (K_phys=1, stationary 2.0,
    nx hi+lo fp8 pair) open each bank start=True, then 4 fp8 DoubleRow
    mains (K_eff=256) accumulate -2 c[m0].x, stop=True.
  units 2..7: psum still holds nx - 2 c[m-1].x after the sqrt CONSUMED it
    (ACT only reads), so 4 DR matmuls with stationary -2(c[m]-c[m-1])
    re-close the banks (start=False, stop=True): nx rides along the whole
    chain and is paid for ONCE instead of per unit. 40 matmuls total, no
    DVE/GpSimd work, no nxb broadcast load.
  per unit: one ACT Sqrt over [128, 2048] with bias=nr[g] (f32 exact) and
    accum_out -> rowsum partial [128, 1] f32, fused in a single pass.
Output per core: racc [128, 8] f32 (m-block x modality rowsums), shipped
as cols 0:7 overlapped with the last unit + col 7 on the tail.
Host: assemble rowsums, compute S pairs + dist_pc in f64, form the loss.

Measured (fast clock state): compute phase is a constant 22.6us from the
first matmul (ACT chain at its 2.03us/unit floor -- sqrt is the only
per-element pass left, DVE/GpSimd idle; PE's 40 MMs at the sustained
427ns fp8-DR cadence hidden under it), bracketed by a ~3.1us DMA-start
latency (brc rides the sync queue first: lands ~0.7us earlier than via
the scalar queue, and it gates the first bias matmul) and a ~10us fixed
framework tail (racc DMA + 253 per-semaphore clears + barriers,
kernel-independent). HW exec 35.3-37.5us across runs (p-state/DMA
jitter) vs 42.8us for the previous DVE-reduce kernel.
Failed probes kept out: 1024-wide matmul dst (walrus s3d3 assert), PE
p-state warmup via dummy matmuls (sustained DR rate is throttle-set, not
ramp-set), in-place sqrt into psum (destroys the delta-chain operand),
tc.tile_wait_until on unit 1 to pin scheduler order (the resulting
schedule DEADLOCKS on hardware -- NRT_EXEC_UNIT_UNRECOVERABLE -- do not
retry; the brx duplicate-parameter dependency achieves the same ordering
safely and keeps ACT0's matmul-count threshold at 8 instead of 12), and
splitting unit 1's sqrt into [128,1024] halves to fill the ACT-queue
hole before the 16-matmul drain (the scheduler interleaves the halves
around later units, delaying the psB chain ~2us: measured +4us).
"""

import sys
from contextlib import ExitStack

import numpy as np

sys.path.insert(0, "/opt/trn_rl_repo")

import concourse.bass as bass
import concourse.tile as tile
from concourse import bacc, mybir
from concourse.bass_utils import run_bass_kernel_spmd

N = 8192
D = 256
HALF = N // 2
NSEG = 1024
NCORES = 8
PW = 8                 # points per label (setup_inputs targets)
GC = 2048              # data columns per core
MB = 4                 # center blocks (of 128) per core
BANK_F = 512           # psum bank width in f32 (matmul dst cap)

FP8 = mybir.dt.float8e4
DR = mybir.MatmulPerfMode.DoubleRow

_nc_cache: dict = {}
last_result = None  # BassKernelResults of the most recent run (for test.py)


def build_nc():
    """One-core SPMD program: fp8 operands -> [128, 8] f32 rowsum shard."""
    f32 = mybir.dt.float32
    bf16 = mybir.dt.bfloat16
    Sqrt = mybir.ActivationFunctionType.Sqrt

    nc = bacc.Bacc()
    # xr[k, t*2048 + i*1024 + j] = x[b*2048 + t*1024 + j, i*128 + k]
    xr_d = nc.declare_dram_parameter("xr", [128, 4096], FP8, isOutput=False)
    # lh[k, mod*1024 + i*512 + g]: g block 0 = -2*c_mod[a*512 + g, 128i+k],
    # g block m>0 = -2*(c[a*512+128m+g] - c[a*512+128(m-1)+g]) (deltas)
    lh_d = nc.declare_dram_parameter("lh", [128, 2048], FP8, isOutput=False)
    # brc[0, 0:256] = 2.0 (bias stationary); [0, 256+i*2048+j]: i=0 ->
    # nx_hi[col]/2, i=1 -> nx_lo[col]/2
    brc_d = nc.declare_dram_parameter("brc", [1, 4352], FP8, isOutput=False)
    # brx: copy of brc whose DMA lands after the xr stream -- unit 1's bias
    # reads it so the scheduler cannot hoist those matmuls ahead of unit 0's
    # mains (that hoist inflates ACT0's matmul-count threshold 8 -> 12)
    brx_d = nc.declare_dram_parameter("brx", [1, 4352], FP8, isOutput=False)
    # nr[p, m_local*2+mod] = ||center[a*512 + m_local*128 + p]||^2 (f32 exact)
    nr_d = nc.declare_dram_parameter("nr", [128, 8], f32, isOutput=False)
    racc_d = nc.declare_dram_parameter("racc", [128, 8], f32, isOutput=True)

    with tile.TileContext(nc) as tc, ExitStack() as ctx:
        const = ctx.enter_context(tc.tile_pool(name="const", bufs=1))
        psum = ctx.enter_context(tc.tile_pool(name="psum", bufs=1, space="PSUM"))

        xr = const.tile([128, 4096], FP8, tag="xr")
        lh = const.tile([128, 2048], FP8, tag="lh")
        brc = const.tile([1, 4352], FP8, tag="brc")
        brx = const.tile([1, 4352], FP8, tag="brx")
        nr_t = const.tile([128, 8], f32, tag="nr")
        warm = const.tile([1, 8], f32, tag="warm")
        racc = const.tile([128, 8], f32, tag="racc")
        d_t = [
            const.tile([128, 2048], bf16, tag=f"d{i}", name=f"d{i}")
            for i in range(2)
        ]
        ps = [
            psum.tile([128, 2048], f32, tag=f"ps{i}", name=f"ps{i}")
            for i in range(2)
        ]

        # brc rides the sync queue FIRST: its packets land ~0.7us earlier
        # than on the scalar queue (whose sequencer is busy with the ACT
        # table loads), and brc gates the first bias matmul = the whole
        # chain's start. xr streams behind it; lh/nr go on the scalar group.
        nc.sync.dma_start(out=brc[:], in_=brc_d[:, :])
        nc.sync.dma_start(out=xr[:, 0:2048], in_=xr_d[:, 0:2048])
        nc.sync.dma_start(out=xr[:, 2048:4096], in_=xr_d[:, 2048:4096])
        nc.sync.dma_start(out=brx[:], in_=brx_d[:, :])
        nc.scalar.dma_start(out=lh[:], in_=lh_d[:, :])
        nc.scalar.dma_start(out=nr_t[:], in_=nr_d[:, :])
        # loads the ACT Sqrt table while the big DMAs are still in flight;
        # fed from brc (the first transfer to land) so the 2x1.3us table
        # loads clear the ACT queue long before the first real sqrt
        nc.scalar.activation(warm[:], brc[:, 0:8], Sqrt)

        def emit_bias(pu, src_t):
            bc3 = src_t[:, 0:256].rearrange("p (i g) -> p i g", i=2)
            br3 = src_t[:, 256:4352].rearrange("p (i n) -> p i n", i=2)
            for t in range(4):
                nc.tensor.matmul(
                    pu[:, t * BANK_F : (t + 1) * BANK_F],
                    bc3,
                    br3[:, :, t * BANK_F : (t + 1) * BANK_F],
                    start=True, stop=False, perf_mode=DR,
                )

        def emit_mains(pu, m_local, mod, first):
            lt = lh[:, mod * 1024 : (mod + 1) * 1024].rearrange(
                "p (i g) -> p i g", i=2
            )[:, :, m_local * 128 : (m_local + 1) * 128]
            xr5 = xr[:].rearrange(
                "p (T i jb j) -> p T jb i j", T=2, i=2, jb=2
            )
            for t in range(4):
                nc.tensor.matmul(
                    pu[:, t * BANK_F : (t + 1) * BANK_F],
                    lt,
                    xr5[:, t // 2, t % 2],
                    start=False, stop=True, perf_mode=DR,
                    skip_group_check=not first,
                )

        def emit_act(pu, u):
            nc.scalar.activation(
                d_t[u % 2][:], pu[:], Sqrt,
                bias=nr_t[:, u : u + 1], scale=1.0,
                accum_out=racc[:, u : u + 1],
            )

        # unit 0 (psA, RGB m0) completes first so the ACT chain starts
        # after 8 real matmuls; unit 1 (psB, IR m0) follows
        emit_bias(ps[0], brc)
        emit_mains(ps[0], 0, 0, first=True)
        emit_act(ps[0], 0)
        emit_bias(ps[1], brx)
        emit_mains(ps[1], 0, 1, first=True)
        emit_act(ps[1], 1)
        # units 2..7: delta mains re-close the still-loaded banks
        for u in range(2, 2 * MB):
            m_local, mod = u // 2, u % 2
            emit_mains(ps[u % 2], m_local, mod, first=False)
            emit_act(ps[u % 2], u)
            if u == 6:
                # most of the output ships while the last unit computes;
                # only the final column rides the critical tail
                nc.sync.dma_start(out=racc_d[:, 0:7], in_=racc[:, 0:7])
        # issued from the ACT engine's own queue: runs in-order right after
        # the last accumulator read, skipping a cross-engine semaphore hop
        nc.scalar.dma_start(out=racc_d[:, 7:8], in_=racc[:, 7:8])
    nc.finalize()
    return nc


def _seg_mean(x_half: np.ndarray, t_half: np.ndarray):
    """f64 segment mean matching jax.ops.segment_sum + max(count,1) divide."""
    cnt = np.bincount(t_half, minlength=NSEG)
    sums = np.zeros((NSEG, D), np.float64)
    order = np.argsort(t_half, kind="stable")
    xs = x_half[order].astype(np.float64)
    ts_sorted = t_half[order]
    present = np.nonzero(cnt)[0]
    if len(present):
        starts = np.searchsorted(ts_sorted, present)
        sums[present] = np.add.reduceat(xs, starts, axis=0)
    return sums / np.maximum(cnt, 1)[:, None], cnt


def prepare(inputs: np.ndarray, targets: np.ndarray):
    """Host marshaling: centers, fp8 DoubleRow operand layouts, in_maps."""
    fp8_np = mybir.dt.np(FP8)
    x = np.asarray(inputs, np.float32)
    t = np.asarray(targets)
    centerR64, _ = _seg_mean(x[:HALF], t[:HALF])
    centerI64, _ = _seg_mean(x[HALF:], t[HALF:])
    centerR = centerR64.astype(np.float32)
    centerI = centerI64.astype(np.float32)
    nrR64 = np.sum(centerR.astype(np.float64) ** 2, axis=1)
    nrI64 = np.sum(centerI.astype(np.float64) ** 2, axis=1)
    n_x64 = np.sum(x.astype(np.float64) ** 2, axis=1)

    cnt_all = np.bincount(t, minlength=NSEG)
    assert cnt_all.min() == cnt_all.max() == PW, "kernel hardcodes 8 pts/label"

    # nx ~= 2*hi + 2*lo with hi, lo in fp8 (e4m3 max 448 forces the /2)
    nxh = (n_x64 / 2.0).astype(fp8_np)
    nxl = ((n_x64 - 2.0 * nxh.astype(np.float64)) / 2.0).astype(fp8_np)

    def mk_lh(center, a):
        c = center[a * 512 : (a + 1) * 512]           # [512, 256]
        w = np.empty((512, D), np.float32)
        w[0:128] = -2.0 * c[0:128]
        for m in range(1, MB):
            w[m * 128 : (m + 1) * 128] = -2.0 * (
                c[m * 128 : (m + 1) * 128] - c[(m - 1) * 128 : m * 128]
            )
        v = w.reshape(512, 2, 128)                    # [g, i, k]
        return np.ascontiguousarray(
            v.transpose(2, 1, 0).reshape(128, 1024)
        ).astype(fp8_np)

    lhs = [
        np.concatenate([mk_lh(centerR, a), mk_lh(centerI, a)], axis=1)
        for a in range(2)
    ]
    nrs = []
    for a in range(2):
        nr_t = np.zeros((128, 8), np.float32)
        for m_local in range(MB):
            sl = slice(a * 512 + m_local * 128, a * 512 + m_local * 128 + 128)
            nr_t[:, m_local * 2] = nrR64[sl]
            nr_t[:, m_local * 2 + 1] = nrI64[sl]
        nrs.append(nr_t)

    in_maps = []
    for c in range(NCORES):
        a, b = c // 4, c % 4
        xc = x[b * GC : (b + 1) * GC]         # [2048, 256], natural order
        xr = np.empty((128, 2, 2, 1024), fp8_np)
        for tt in range(2):
            v = xc[tt * 1024 : (tt + 1) * 1024].reshape(1024, 2, 128)
            xr[:, tt] = v.transpose(2, 1, 0)  # [k, i, j]
        brc = np.empty((1, 4352), fp8_np)
        brc[0, :256] = np.float32(2.0)
        brc[0, 256 : 256 + 2048] = nxh[b * GC : (b + 1) * GC]
        brc[0, 256 + 2048 :] = nxl[b * GC : (b + 1) * GC]
        in_maps.append(
            {
                "xr": np.ascontiguousarray(xr.reshape(128, 4096)),
                "lh": lhs[a],
                "brc": brc,
                "brx": brc,
                "nr": nrs[a],
            }
        )

    host = dict(
        centerR64=centerR64, centerI64=centerI64,
        nrR64=nrR64, nrI64=nrI64, n_x64=n_x64,
        cnt_all=cnt_all, targets=t, x=x,
    )
    return in_maps, host


def finish(core_outs, host) -> np.float32:
    """Assemble rowsums; S pairs + dist_pc exactly in f64; form the loss."""
    t = host["targets"]
    cnt = host["cnt_all"]
    cR, cI = host["centerR64"], host["centerI64"]
    nrR, nrI = host["nrR64"], host["nrI64"]
    nx = host["n_x64"]
    x = host["x"].astype(np.float64)

    rowsumR = np.zeros(NSEG, np.float64)
    rowsumI = np.zeros(NSEG, np.float64)
    for c in range(NCORES):
        a = c // 4
        racc = core_outs[c].astype(np.float64)    # [128, 8]
        for m_local in range(MB):
            rows = slice(a * 512 + m_local * 128, a * 512 + m_local * 128 + 128)
            rowsumR[rows] += racc[:, m_local * 2]
            rowsumI[rows] += racc[:, m_local * 2 + 1]

    # masked-out terms S[g, h] = sum_{j: t_j==h} dist(c_g, x_j), exact f64,
    # only for the (row center, row label) pairs the reference subtracts
    gqR = t[np.arange(N) % HALF]
    gqI = t[HALF + (np.arange(N) % HALF)]
    order = np.argsort(t, kind="stable")
    pts_by_label = x[order].reshape(NSEG, PW, D)      # label-major points
    nx_by_label = nx[order].reshape(NSEG, PW)

    def s_pairs(gq, centers, nrc):
        pair_ids = gq.astype(np.int64) * NSEG + t
        uniq, inv = np.unique(pair_ids, return_inverse=True)
        ug, uh = uniq // NSEG, uniq % NSEG
        cc = centers[ug]                               # [u, D]
        pp = pts_by_label[uh]                          # [u, PW, D]
        d2 = (
            nrc[ug][:, None]
            + nx_by_label[uh]
            - 2.0 * np.einsum("ud,upd->up", cc, pp)
        )
        s_u = np.sqrt(np.maximum(d2, 1e-12)).sum(axis=1)
        return s_u[inv]                                # [N]

    SR = s_pairs(gqR, cR, nrR)
    SI = s_pairs(gqI, cI, nrI)
    a_w = 1.0 / (N - cnt[t]).astype(np.float64)
    sumR = float(np.sum(a_w * (rowsumR[gqR] - SR)))
    sumI = float(np.sum(a_w * (rowsumI[gqI] - SI)))

    diff = cR[t[:HALF]] - cI[t[HALF:]]
    s_pc = float(np.sum(np.sqrt(np.sum(diff * diff, axis=1))))
    return np.float32(s_pc / (sumR + sumI - s_pc))


def kernel(inputs: np.ndarray, targets: np.ndarray) -> np.ndarray:
    global last_result
    in_maps, host = prepare(inputs, targets)
    if "nc" not in _nc_cache:
        _nc_cache["nc"] = build_nc()
    nc = _nc_cache["nc"]
    res = run_bass_kernel_spmd(nc, in_maps, list(range(NCORES)))
    last_result = res
    outs = [res.results[c]["racc"] for c in range(NCORES)]
    return finish(outs, host)
